# revision 12
# baseline (speedup 1.0000x reference)
"""Trainium2 Bass kernel for nn_CLFBlock (linear -> LIF scan -> linear -> T-mean -> log_softmax).

Self-contained: hardcodes shapes T=32, B=512, D=1024, C=1000 and data-parallel
sharding of the batch dim across 8 NeuronCores.

Math notes:
  h = x @ W1.T + b1                      (computed in bf16 on the PE, fp32 accum)
  LIF (tau=2, v_th=1, hard reset to 0):
     v' = 0.5*v + 0.5*h
     s  = (v' >= 1);  v = v' * (v' < 1)
  Scan state is kept pre-halved:  hh = 0.5*h + 0.5*b1,  vh = 0.5*v, so per step
     w  = vh + hh            (tensor_tensor add, 2x DVE mode)
     mh = (w < 1) * 0.5      (one fused tensor_scalar, 4x DVE mode)
     vh = w * mh             (tensor_tensor mult, 2x DVE mode)
  Spike sum accumulates on the tensor engine: msum_psum += I @ mh_t, and
  sum_t s_t = T - 2*msum.
  y = mean_t(s_t @ W2.T + b2) = (sum_t s_t) @ W2.T / T + b2
  out = log_softmax(y, axis=1)

Layout: the tensor engine contracts along the partition axis, so both matmul
operands need the contraction dim (d / e) on partitions. The host-side shard
step hands each core its x slice already transposed ([D, T*Bc]) and the
weights transposed once ([D, D] / [D, C]), so the device needs no transposes
at all: inputs are cast fp32->bf16 during the HBM load (SWDGE cast-DMA) into
their final layouts. mm1 emission is interleaved with the LIF scan so the
tensor engine stream k0,k1,ms0-7,k2,ms8-15,k3,ms16-31 stays dense and warm.
"""

import numpy as np
from contextlib import ExitStack

import concourse.bass as bass
import concourse.tile as tile
from concourse import bacc, mybir
from concourse.bass_utils import run_bass_kernel_spmd

N_CORES = 8
T, B, D, C = 32, 512, 1024, 1000
BC = B // N_CORES          # 64 rows per core
TB = T * BC                # 2048 matmul rows per core
FP32 = mybir.dt.float32
BF16 = mybir.dt.bfloat16
AF = mybir.ActivationFunctionType
OP = mybir.AluOpType


def build_program():
    nc = bacc.Bacc("TRN2", target_bir_lowering=False, debug=False, num_devices=N_CORES)

    xt_d = nc.dram_tensor("xT", [D, TB], FP32, kind="ExternalInput").ap()
    w1t_d = nc.dram_tensor("W1T", [D, D], FP32, kind="ExternalInput").ap()
    b1_d = nc.dram_tensor("b1", [D], FP32, kind="ExternalInput").ap()
    w2t_d = nc.dram_tensor("W2T", [D, C], FP32, kind="ExternalInput").ap()
    b2_d = nc.dram_tensor("b2", [C], FP32, kind="ExternalInput").ap()
    y_d = nc.dram_tensor("y", [BC, C], FP32, kind="ExternalOutput").ap()

    with tile.TileContext(nc) as tc, ExitStack() as ctx:
        persist = ctx.enter_context(tc.tile_pool(name="persist", bufs=1))
        mpool = ctx.enter_context(tc.tile_pool(name="mpool", bufs=T))
        small = ctx.enter_context(tc.tile_pool(name="small", bufs=1))
        ps_h = ctx.enter_context(tc.tile_pool(name="ps_h", bufs=4, space="PSUM"))
        ps_ms = ctx.enter_context(tc.tile_pool(name="ps_ms", bufs=1, space="PSUM"))
        ps_y = ctx.enter_context(tc.tile_pool(name="ps_y", bufs=2, space="PSUM"))

        # ---- constants / biases needed early ----
        io = small.tile([128, 128], mybir.dt.int32)
        nc.gpsimd.iota(io[:], pattern=[[1, 128]], base=0, channel_multiplier=-1)
        ident = small.tile([128, 128], BF16)
        nc.vector.tensor_scalar(ident[:], io[:], 0, None, op0=OP.is_equal)

        b1_sb = small.tile([128, 8], FP32)
        nc.scalar.dma_start(b1_sb[:], b1_d.rearrange("(j p) -> p j", p=128))
        b1h = small.tile([128, 8], FP32)
        nc.vector.tensor_scalar_mul(b1h[:], b1_sb[:], 0.5)

        # warm the ACT Exp/Ln spline tables during the prologue so the
        # epilogue doesn't pay the ~2.7us table-load switch
        warm = small.tile([1, 8], FP32)
        nc.scalar.activation(warm[:, 0:4], b1_sb[0:1, 0:4], AF.Exp)
        nc.scalar.activation(warm[:, 4:8], b1_sb[0:1, 4:8], AF.Ln)

        # ---- W1T: plain fp32 load (sync queue) + DVE cast -> w1t bf16 ----
        stage = ctx.enter_context(tc.tile_pool(name="stage", bufs=3))
        w1t = persist.tile([128, 8 * 1024], BF16)
        w1t3 = w1t[:].rearrange("p (j e) -> p j e", j=8)
        for ci in range(2):
            w1f = stage.tile([128, 4 * 1024], FP32, tag="stage", name=f"w1f{ci}")
            nc.sync.dma_start(
                w1f[:].rearrange("p (j e) -> p j e", j=4),
                w1t_d[ci * 512:(ci + 1) * 512, :].rearrange("(dj p) e -> p dj e", p=128),
            )
            nc.vector.tensor_copy(w1t3[:, 4 * ci:4 * (ci + 1), :],
                                  w1f[:].rearrange("p (j e) -> p j e", j=4))

        # ---- xT: plain fp32 load (scalar queue) + DVE cast -> xt bf16 ----
        xt = persist.tile([128, 8 * TB], BF16)
        xt3 = xt[:].rearrange("p (j t) -> p j t", j=8)

        def load_x_chunk(k):
            xf = stage.tile([128, 4 * 1024], FP32, tag="stage", name=f"xf{k}")
            nc.scalar.dma_start(
                xf[:].rearrange("p (j t) -> p j t", j=8),
                xt_d[:, k * 512:(k + 1) * 512].rearrange("(dj p) t -> p dj t", p=128),
            )
            nc.vector.tensor_copy(xt3[:, :, k * 512:(k + 1) * 512],
                                  xf[:].rearrange("p (j t) -> p j t", j=8))

        load_x_chunk(0)
        load_x_chunk(1)

        # ---- matmul1: h[e, tb] = W1 @ x.T, fused 0.5*h + 0.5*b1 into scan layout ----
        # h_sb free index = t*512 + j*64 + b
        h_sb = persist.tile([128, T * 512], BF16)
        h3 = h_sb[:].rearrange("p (t x) -> p t x", x=512)

        def mm1_group(k):
            for j in range(8):
                ps = ps_h.tile([128, 512], FP32, tag="ps_h", name=f"psh_{k}_{j}")
                for di in range(8):
                    nc.tensor.matmul(
                        ps[:],
                        w1t[:, di * 1024 + j * 128: di * 1024 + (j + 1) * 128],
                        xt[:, di * TB + k * 512: di * TB + (k + 1) * 512],
                        start=(di == 0), stop=(di == 7),
                    )
                nc.scalar.activation(
                    h3[:, 8 * k:8 * k + 8, j * 64:(j + 1) * 64],
                    ps[:].rearrange("p (t b) -> p t b", t=8),
                    AF.Identity, scale=0.5, bias=b1h[:, j:j + 1],
                )

        # ---- LIF scan pieces (emitted interleaved with mm1 groups) ----
        vh = small.tile([128, 512], BF16)   # 0.5 * v
        w = small.tile([128, 512], BF16)
        nc.vector.memset(vh[:], 0.0)
        msum = ps_ms.tile([128, 512], FP32)

        def scan_steps(t0, t1):
            for t in range(t0, t1):
                h_t = h_sb[:, t * 512:(t + 1) * 512]
                nc.vector.tensor_add(w[:], vh[:], h_t)
                m = mpool.tile([128, 512], BF16, tag="m", name=f"m{t}")
                nc.vector.tensor_scalar(m[:], w[:], 1.0, 0.5, op0=OP.is_lt, op1=OP.mult)
                nc.vector.tensor_mul(vh[:], w[:], m[:])
                nc.tensor.matmul(msum[:], ident[:], m[:],
                                 start=(t == 0), stop=(t == T - 1))

        mm1_group(0)
        load_x_chunk(2)
        load_x_chunk(3)
        mm1_group(1)
        scan_steps(0, 8)
        mm1_group(2)
        scan_steps(8, 16)
        mm1_group(3)
        scan_steps(16, 32)

        # sum_t s_t = T - 2 * msum
        ssum = small.tile([128, 512], BF16)
        nc.scalar.activation(ssum[:], msum[:], AF.Copy, scale=-2.0, bias=float(T))

        # ---- W2T (only needed now): cast-load -> w2t[ep, ej*1024 + c] ----
        w2t = persist.tile([128, 8 * 1024], BF16)
        w2t3 = w2t[:].rearrange("p (j c) -> p j c", j=8)
        for ci in range(2):
            nc.gpsimd.dma_start(
                w2t3[:, 4 * ci:4 * (ci + 1), 0:C],
                w2t_d[ci * 512:(ci + 1) * 512, :].rearrange("(ej p) c -> p ej c", p=128),
            )

        b2_sb = small.tile([1, C], FP32)
        nc.scalar.dma_start(b2_sb[:], b2_d.rearrange("(a c) -> a c", a=1))
        b2_32 = small.tile([1, C], BF16)
        nc.scalar.activation(b2_32[:], b2_sb[:], AF.Copy, scale=float(T))
        ones = small.tile([1, BC], BF16)
        nc.vector.memset(ones[:], 1.0)

        # ---- matmul2: y = ssum @ W2.T / T + b2 ----
        y_sb = small.tile([BC, 1024], FP32)
        for half in range(2):
            n = 512 if half == 0 else C - 512
            c0 = half * 512
            psy = ps_y.tile([BC, 512], FP32, tag="ps_y", name=f"psy{half}")
            for ej in range(8):
                nc.tensor.matmul(
                    psy[:, 0:n],
                    ssum[:, ej * 64:(ej + 1) * 64],
                    w2t[:, ej * 1024 + c0: ej * 1024 + c0 + n],
                    start=(ej == 0), stop=False,
                )
            nc.tensor.matmul(psy[:, 0:n], ones[:], b2_32[:, c0:c0 + n],
                             start=False, stop=True)
            nc.scalar.activation(y_sb[:, c0:c0 + n], psy[:, 0:n], AF.Copy,
                                 scale=1.0 / T)

        # ---- log_softmax over C ----
        mx = small.tile([BC, 1], FP32)
        nc.vector.reduce_max(mx[:], y_sb[:, 0:C], axis=mybir.AxisListType.X)
        z = small.tile([BC, 1024], FP32)
        nc.vector.tensor_scalar(z[:, 0:C], y_sb[:, 0:C], mx[:], None, op0=OP.subtract)
        ez = small.tile([BC, 1024], FP32)
        nc.scalar.activation(ez[:, 0:C], z[:, 0:C], AF.Exp)
        ssum_e = small.tile([BC, 1], FP32)
        nc.vector.reduce_sum(ssum_e[:], ez[:, 0:C], axis=mybir.AxisListType.X)
        lse = small.tile([BC, 1], FP32)
        nc.scalar.activation(lse[:], ssum_e[:], AF.Ln)
        out_sb = small.tile([BC, C], FP32)
        nc.vector.tensor_scalar(out_sb[:], z[:, 0:C], lse[:], None, op0=OP.subtract)
        nc.sync.dma_start(y_d[:], out_sb[:])

    nc.compile()
    return nc


_CACHE = {}


def kernel(x, W1, b1, W2, b2):
    if "nc" not in _CACHE:
        _CACHE["nc"] = build_program()
    nc = _CACHE["nc"]

    x = np.asarray(x, dtype=np.float32)
    w1t = np.ascontiguousarray(np.asarray(W1, dtype=np.float32).T)   # [D, D]
    w2t = np.ascontiguousarray(np.asarray(W2, dtype=np.float32).T)   # [D, C]
    b1 = np.ascontiguousarray(b1, dtype=np.float32)
    b2 = np.ascontiguousarray(b2, dtype=np.float32)
    in_maps = []
    for i in range(N_CORES):
        xs = np.ascontiguousarray(
            x[:, i * BC:(i + 1) * BC, :].reshape(TB, D).T)           # [D, TB]
        in_maps.append({"xT": xs, "W1T": w1t, "b1": b1, "W2T": w2t, "b2": b2})

    res = run_bass_kernel_spmd(nc, in_maps, core_ids=list(range(N_CORES)),
                               **_CACHE.get("run_kwargs", {}))
    _CACHE["last_results"] = res
    out = np.concatenate([res.results[i]["y"] for i in range(N_CORES)], axis=0)
    return out


# revision 13
# speedup vs baseline: 1.1759x; 1.1759x over previous
"""Trainium2 Bass kernel for nn_CLFBlock (linear -> LIF scan -> linear -> T-mean -> log_softmax).

Self-contained: hardcodes shapes T=32, B=512, D=1024, C=1000 and data-parallel
sharding of the batch dim across 8 NeuronCores.

Math notes:
  h = x @ W1.T + b1                      (computed in bf16 on the PE, fp32 accum)
  LIF (tau=2, v_th=1, hard reset to 0):
     v' = 0.5*v + 0.5*h
     s  = (v' >= 1);  v = v' * (v' < 1)
  Scan state is kept pre-halved:  hh = 0.5*h + 0.5*b1,  vh = 0.5*v, so per step
     w  = vh + hh            (tensor_tensor add, 2x DVE mode)
     mh = (w < 1) * 0.5      (one fused tensor_scalar, 4x DVE mode)
     vh = w * mh             (tensor_tensor mult, 2x DVE mode)
  Spike sum accumulates on the tensor engine: msum_psum += I @ mh_t, and
  sum_t s_t = T - 2*msum.
  y = mean_t(s_t @ W2.T + b2) = (sum_t s_t) @ W2.T / T + b2
  out = log_softmax(y, axis=1)

Layout: the tensor engine contracts along the partition axis, so both matmul
operands need the contraction dim (d / e) on partitions. The host-side shard
step hands each core its x slice already transposed ([D, T*Bc]) and the
weights transposed once ([D, D] / [D, C]), so the device needs no transposes
at all: inputs are cast fp32->bf16 during the HBM load (SWDGE cast-DMA) into
their final layouts. mm1 emission is interleaved with the LIF scan so the
tensor engine stream k0,k1,ms0-7,k2,ms8-15,k3,ms16-31 stays dense and warm.
"""

import numpy as np
from contextlib import ExitStack

import concourse.bass as bass
import concourse.tile as tile
from concourse import bacc, mybir
from concourse.bass_utils import run_bass_kernel_spmd

N_CORES = 8
T, B, D, C = 32, 512, 1024, 1000
BC = B // N_CORES          # 64 rows per core
TB = T * BC                # 2048 matmul rows per core
FP32 = mybir.dt.float32
BF16 = mybir.dt.bfloat16
AF = mybir.ActivationFunctionType
OP = mybir.AluOpType


def build_program():
    nc = bacc.Bacc("TRN2", target_bir_lowering=False, debug=False, num_devices=N_CORES)

    xt_d = nc.dram_tensor("xT", [D, TB], BF16, kind="ExternalInput").ap()
    w1t_d = nc.dram_tensor("W1T", [D, D], BF16, kind="ExternalInput").ap()
    b1_d = nc.dram_tensor("b1", [D], FP32, kind="ExternalInput").ap()
    w2t_d = nc.dram_tensor("W2T", [D, C], BF16, kind="ExternalInput").ap()
    b2_d = nc.dram_tensor("b2", [C], FP32, kind="ExternalInput").ap()
    y_d = nc.dram_tensor("y", [BC, C], FP32, kind="ExternalOutput").ap()

    with tile.TileContext(nc) as tc, ExitStack() as ctx:
        persist = ctx.enter_context(tc.tile_pool(name="persist", bufs=1))
        mpool = ctx.enter_context(tc.tile_pool(name="mpool", bufs=T))
        small = ctx.enter_context(tc.tile_pool(name="small", bufs=1))
        ps_h = ctx.enter_context(tc.tile_pool(name="ps_h", bufs=4, space="PSUM"))
        ps_ms = ctx.enter_context(tc.tile_pool(name="ps_ms", bufs=1, space="PSUM"))
        ps_y = ctx.enter_context(tc.tile_pool(name="ps_y", bufs=2, space="PSUM"))

        # ---- constants / biases needed early ----
        io = small.tile([128, 128], mybir.dt.int32)
        nc.gpsimd.iota(io[:], pattern=[[1, 128]], base=0, channel_multiplier=-1)
        ident = small.tile([128, 128], BF16)
        nc.vector.tensor_scalar(ident[:], io[:], 0, None, op0=OP.is_equal)

        b1_sb = small.tile([128, 8], FP32)
        nc.scalar.dma_start(b1_sb[:], b1_d.rearrange("(j p) -> p j", p=128))
        b1h = small.tile([128, 8], FP32)
        nc.vector.tensor_scalar_mul(b1h[:], b1_sb[:], 0.5)

        # warm the ACT Exp/Ln spline tables during the prologue so the
        # epilogue doesn't pay the ~2.7us table-load switch
        warm = small.tile([1, 8], FP32)
        nc.scalar.activation(warm[:, 0:4], b1_sb[0:1, 0:4], AF.Exp)
        nc.scalar.activation(warm[:, 4:8], b1_sb[0:1, 4:8], AF.Ln)

        # ---- W1T: direct bf16 load (sync queue) ----
        w1t = persist.tile([128, 8 * 1024], BF16)
        w1t3 = w1t[:].rearrange("p (j e) -> p j e", j=8)
        nc.sync.dma_start(
            w1t3[:],
            w1t_d[:].rearrange("(dj p) e -> p dj e", p=128),
        )

        # ---- xT: direct bf16 loads (scalar queue) ----
        xt = persist.tile([128, 8 * TB], BF16)
        xt3 = xt[:].rearrange("p (j t) -> p j t", j=8)

        def load_x_chunk(k):
            nc.scalar.dma_start(
                xt3[:, :, k * 512:(k + 1) * 512],
                xt_d[:, k * 512:(k + 1) * 512].rearrange("(dj p) t -> p dj t", p=128),
            )

        load_x_chunk(0)
        load_x_chunk(1)

        # ---- matmul1: h[e, tb] = W1 @ x.T, fused 0.5*h + 0.5*b1 into scan layout ----
        # h_sb free index = t*512 + j*64 + b
        h_sb = persist.tile([128, T * 512], BF16)
        h3 = h_sb[:].rearrange("p (t x) -> p t x", x=512)

        def mm1_group(k):
            for j in range(8):
                ps = ps_h.tile([128, 512], FP32, tag="ps_h", name=f"psh_{k}_{j}")
                for di in range(8):
                    nc.tensor.matmul(
                        ps[:],
                        w1t[:, di * 1024 + j * 128: di * 1024 + (j + 1) * 128],
                        xt[:, di * TB + k * 512: di * TB + (k + 1) * 512],
                        start=(di == 0), stop=(di == 7),
                    )
                nc.scalar.activation(
                    h3[:, 8 * k:8 * k + 8, j * 64:(j + 1) * 64],
                    ps[:].rearrange("p (t b) -> p t b", t=8),
                    AF.Identity, scale=0.5, bias=b1h[:, j:j + 1],
                )

        # ---- LIF scan pieces (emitted interleaved with mm1 groups) ----
        vh = small.tile([128, 512], BF16)   # 0.5 * v
        w = small.tile([128, 512], BF16)
        nc.vector.memset(vh[:], 0.0)
        msum = ps_ms.tile([128, 512], FP32)

        def scan_steps(t0, t1):
            for t in range(t0, t1):
                h_t = h_sb[:, t * 512:(t + 1) * 512]
                nc.vector.tensor_add(w[:], vh[:], h_t)
                m = mpool.tile([128, 512], BF16, tag="m", name=f"m{t}")
                nc.vector.tensor_scalar(m[:], w[:], 1.0, 0.5, op0=OP.is_lt, op1=OP.mult)
                nc.vector.tensor_mul(vh[:], w[:], m[:])
                nc.tensor.matmul(msum[:], ident[:], m[:],
                                 start=(t == 0), stop=(t == T - 1))

        mm1_group(0)
        load_x_chunk(2)
        load_x_chunk(3)
        mm1_group(1)
        scan_steps(0, 8)
        mm1_group(2)
        scan_steps(8, 16)
        mm1_group(3)
        scan_steps(16, 32)

        # sum_t s_t = T - 2 * msum
        ssum = small.tile([128, 512], BF16)
        nc.scalar.activation(ssum[:], msum[:], AF.Copy, scale=-2.0, bias=float(T))

        # ---- W2T (only needed now): direct bf16 load ----
        w2t = persist.tile([128, 8 * 1024], BF16)
        w2t3 = w2t[:].rearrange("p (j c) -> p j c", j=8)
        nc.sync.dma_start(
            w2t3[:, :, 0:C],
            w2t_d[:].rearrange("(ej p) c -> p ej c", p=128),
        )

        b2_sb = small.tile([1, C], FP32)
        nc.scalar.dma_start(b2_sb[:], b2_d.rearrange("(a c) -> a c", a=1))
        b2_32 = small.tile([1, C], BF16)
        nc.scalar.activation(b2_32[:], b2_sb[:], AF.Copy, scale=float(T))
        ones = small.tile([1, BC], BF16)
        nc.vector.memset(ones[:], 1.0)

        # ---- matmul2: y = ssum @ W2.T / T + b2 ----
        y_sb = small.tile([BC, 1024], FP32)
        for half in range(2):
            n = 512 if half == 0 else C - 512
            c0 = half * 512
            psy = ps_y.tile([BC, 512], FP32, tag="ps_y", name=f"psy{half}")
            for ej in range(8):
                nc.tensor.matmul(
                    psy[:, 0:n],
                    ssum[:, ej * 64:(ej + 1) * 64],
                    w2t[:, ej * 1024 + c0: ej * 1024 + c0 + n],
                    start=(ej == 0), stop=False,
                )
            nc.tensor.matmul(psy[:, 0:n], ones[:], b2_32[:, c0:c0 + n],
                             start=False, stop=True)
            nc.scalar.activation(y_sb[:, c0:c0 + n], psy[:, 0:n], AF.Copy,
                                 scale=1.0 / T)

        # ---- log_softmax over C ----
        mx = small.tile([BC, 1], FP32)
        nc.vector.reduce_max(mx[:], y_sb[:, 0:C], axis=mybir.AxisListType.X)
        z = small.tile([BC, 1024], FP32)
        nc.vector.tensor_scalar(z[:, 0:C], y_sb[:, 0:C], mx[:], None, op0=OP.subtract)
        ez = small.tile([BC, 1024], FP32)
        nc.scalar.activation(ez[:, 0:C], z[:, 0:C], AF.Exp)
        ssum_e = small.tile([BC, 1], FP32)
        nc.vector.reduce_sum(ssum_e[:], ez[:, 0:C], axis=mybir.AxisListType.X)
        lse = small.tile([BC, 1], FP32)
        nc.scalar.activation(lse[:], ssum_e[:], AF.Ln)
        out_sb = small.tile([BC, C], FP32)
        nc.vector.tensor_scalar(out_sb[:], z[:, 0:C], lse[:], None, op0=OP.subtract)
        nc.sync.dma_start(y_d[:], out_sb[:])

    nc.compile()
    return nc


_CACHE = {}


def kernel(x, W1, b1, W2, b2):
    if "nc" not in _CACHE:
        _CACHE["nc"] = build_program()
    nc = _CACHE["nc"]

    import ml_dtypes
    x = np.asarray(x, dtype=np.float32)
    w1t = np.ascontiguousarray(np.asarray(W1, dtype=np.float32).T.astype(ml_dtypes.bfloat16))
    w2t = np.ascontiguousarray(np.asarray(W2, dtype=np.float32).T.astype(ml_dtypes.bfloat16))
    b1 = np.ascontiguousarray(b1, dtype=np.float32)
    b2 = np.ascontiguousarray(b2, dtype=np.float32)
    in_maps = []
    for i in range(N_CORES):
        xs = np.ascontiguousarray(
            x[:, i * BC:(i + 1) * BC, :].reshape(TB, D).T.astype(ml_dtypes.bfloat16))
        in_maps.append({"xT": xs, "W1T": w1t, "b1": b1, "W2T": w2t, "b2": b2})

    res = run_bass_kernel_spmd(nc, in_maps, core_ids=list(range(N_CORES)),
                               **_CACHE.get("run_kwargs", {}))
    _CACHE["last_results"] = res
    out = np.concatenate([res.results[i]["y"] for i in range(N_CORES)], axis=0)
    return out
